# revision 11
# baseline (speedup 1.0000x reference)
"""Trainium2 Bass kernel for MQA causal attention — v4.

Sharding: hybrid batch x tensor-parallel (2 batches x 4 head-slices on 8
cores); shared K/V computed locally per batch; host sums 4 partial
out-projections per batch.

v4 over v3:
  - attention processes key chunks in PAIRS: two sim matmuls land in one
    2-bank PSUM tile, ONE exp instruction covers both (halves ACT fixed
    cost), a DVE pair-sum feeds one denominator matmul per pair (halves
    the ones-column PE cost)
  - causal masking via constant 0/1 mask tiles + DVE multiply (gpsimd
    affine_select leaves the per-chunk critical chain; Pool only does the
    per-tile denominator broadcast and y DMA triggers)
  - rope runs in bf16 on DVE (2x mode) after an ACT-side PSUM drain
  - partial y written in bf16 (halves output DMA; host upcasts)
  - 2-pair software pipelining keeps PE fed while ACT runs exp
"""

import os
import sys
from contextlib import ExitStack

import numpy as np

for _p in ("/opt/trn_rl_repo",):
    if os.path.isdir(_p) and _p not in sys.path:
        sys.path.insert(0, _p)

import ml_dtypes

import concourse.bass as bass
import concourse.mybir as mybir
import concourse.tile as tile
from concourse import bacc
from concourse.bass_utils import run_bass_kernel_spmd
from concourse.masks import make_identity

HEADS = 16
D = 128
SCALE = D ** -0.5
N_CORES = 8
HPC = 4              # query heads per core (hybrid: 2 batches x 4 head slices)
GROUPS = HPC // 2    # attention processed in 2-head groups (PSUM budget)

F32 = mybir.dt.float32
BF16 = mybir.dt.bfloat16


def _rope(nc, sb_pool, ps, out_slice, cos_s, sin_s):
    """out_slice(bf16) = ps*cos_s + rot(ps)*sin_s. ACT performs the
    rotate-half during the PSUM drain (partition-offset copies are legal
    with a PSUM operand; SBUF-SBUF DVE ops must be partition-aligned), so
    the sin multiply and the add run in bf16 2x mode on DVE; sin_s arrives
    pre-signed from the host (rows 0-63 negated)."""
    L = ps.shape[-1]
    pr = sb_pool.tile([128, L], BF16, tag="ropepr")
    nc.scalar.copy(pr[0:64, :], ps[64:128, :])
    nc.scalar.copy(pr[64:128, :], ps[0:64, :])
    t1 = sb_pool.tile([128, L], BF16, tag="ropet1")
    nc.vector.tensor_mul(t1, ps, cos_s)
    t2 = sb_pool.tile([128, L], BF16, tag="ropet2")
    nc.vector.tensor_mul(t2, pr, sin_s)
    nc.vector.tensor_add(out_slice, t1, t2)


def build_nc(B, N, DIM, HL, reps=1):
    """One SPMD program: HL query heads + shared kv head, ONE batch."""
    DC = DIM // 128           # contraction chunks for projections
    SL = min(N, 512)          # projection n-slice length
    NS = N // SL              # n slices
    NKC = N // 128            # 128-wide key chunks
    NQT = N // 256            # 256-row query tiles
    KPS = SL // 128           # key chunks per slice

    nc = bacc.Bacc(None, target_bir_lowering=False)
    xT = nc.declare_dram_parameter("xT", [DIM, N], BF16, isOutput=False)
    wq = nc.declare_dram_parameter("wq", [DIM, HL * D], BF16, isOutput=False)
    wkv = nc.declare_dram_parameter("wkv", [DIM, 2 * D], BF16, isOutput=False)
    wout = nc.declare_dram_parameter("wout", [HL * D, DIM], BF16, isOutput=False)
    cosq = nc.declare_dram_parameter("cosq", [D, N], BF16, isOutput=False)
    sinq = nc.declare_dram_parameter("sinq", [D, N], BF16, isOutput=False)
    cosk = nc.declare_dram_parameter("cosk", [D, N], BF16, isOutput=False)
    sink = nc.declare_dram_parameter("sink", [D, N], BF16, isOutput=False)
    y = nc.declare_dram_parameter("y", [N, DIM], BF16, isOutput=True)

    with ExitStack() as ctx:
        tc = ctx.enter_context(tile.TileContext(nc))
        consts = ctx.enter_context(tc.tile_pool(name="consts", bufs=1))
        xpool = ctx.enter_context(tc.tile_pool(name="xpool", bufs=2))
        proj = ctx.enter_context(tc.tile_pool(name="proj", bufs=2))
        sb = ctx.enter_context(tc.tile_pool(name="sb", bufs=2))
        expool = ctx.enter_context(tc.tile_pool(name="expool", bufs=3))
        asb = ctx.enter_context(tc.tile_pool(name="asb", bufs=3))
        outp = ctx.enter_context(tc.tile_pool(name="outp", bufs=2))
        ps_work = ctx.enter_context(tc.tile_pool(name="ps_work", bufs=2, space="PSUM"))
        ps_att = ctx.enter_context(tc.tile_pool(name="ps_att", bufs=2, space="PSUM"))
        # psd is read (reciprocal) immediately after its stop, so one buf
        # suffices; the freed bank gives outproj psy its own pool so drained
        # column-groups never collide with the sim-pair buffer rotation
        ps_den = ctx.enter_context(tc.tile_pool(name="ps_den", bufs=1, space="PSUM"))
        ps_out = ctx.enter_context(tc.tile_pool(name="ps_out", bufs=1, space="PSUM"))

        ident = consts.tile([128, 128], BF16)
        make_identity(nc, ident)
        ones_col = consts.tile([128, 1], BF16)
        nc.vector.memset(ones_col, 1.0)
        # causal 0/1 masks for the two diagonal key chunks of each q tile:
        # chunk aligned with queries (keep q >= p) and chunk 128 past it
        masks = []
        for base in (0, -128):
            mk = consts.tile([128, 2, 256], BF16, name=f"mask{base}")
            nc.vector.memset(mk, 1.0)
            nc.gpsimd.affine_select(
                out=mk, in_=mk, compare_op=mybir.AluOpType.is_ge, fill=0.0,
                base=base, pattern=[[0, 2], [1, 256]], channel_multiplier=-1)
            masks.append(mk)

        wq_sb = consts.tile([128, DC, HL * D], BF16)
        wkv_sb = consts.tile([128, DC, 2 * D], BF16)
        nc.sync.dma_start(
            wq_sb, wq.rearrange("(c p) m -> p c m", p=128))
        nc.sync.dma_start(
            wkv_sb, wkv.rearrange("(c p) m -> p c m", p=128))
        wout_sb = consts.tile([128, HL, DIM], BF16)
        nc.scalar.dma_start(wout_sb, wout.rearrange("(c p) m -> p c m", p=128))
        cq_sb = consts.tile([128, N], BF16)
        sq_sb = consts.tile([128, N], BF16)
        ck_sb = consts.tile([128, N], BF16)
        sk_sb = consts.tile([128, N], BF16)
        nc.scalar.dma_start(cq_sb, cosq[:, :])
        nc.scalar.dma_start(sq_sb, sinq[:, :])
        nc.scalar.dma_start(ck_sb, cosk[:, :])
        nc.scalar.dma_start(sk_sb, sink[:, :])

        for rep in range(reps):
            qrot = proj.tile([128, HL, N], BF16, tag="qrot")
            krot = proj.tile([128, N], BF16, tag="krot")
            vnat = proj.tile([128, NKC, D], BF16, tag="vnat")
            attnT = proj.tile([128, HL, N], BF16, tag="attnT")

            def _attn_qtile(t, g, qrot=qrot, krot=krot, vnat=vnat, attnT=attnT):
                h0, h1 = 2 * g, 2 * g + 2
                P = t + 1                     # key-chunk pairs (2t+2 chunks)
                psa = ps_att.tile([128, 2, 256], F32, tag="psa")
                psd = ps_den.tile([1, 2, 256], F32, tag="psd")
                qsl = qrot[:, h0:h1, t * 256:(t + 1) * 256]

                # two sims + one exp + one pair-sum per PAIR of key chunks;
                # issued 2 pairs ahead of the consuming psd/psa matmuls so
                # the PE never waits on ACT in program order
                def _simpair(p):
                    pss = ps_work.tile([128, 2, 2, 256], F32, tag="pswork")
                    nc.tensor.matmul(
                        pss[:, 0], krot[:, (2 * p) * 128:(2 * p + 1) * 128],
                        qsl, start=True, stop=True)
                    nc.tensor.matmul(
                        pss[:, 1], krot[:, (2 * p + 1) * 128:(2 * p + 2) * 128],
                        qsl, start=True, stop=True)
                    ex = expool.tile([128, 2, 2, 256], BF16, tag="exp")
                    nc.scalar.activation(ex, pss, mybir.ActivationFunctionType.Exp)
                    if p == t:            # diagonal pair
                        nc.vector.tensor_mul(ex[:, 0], ex[:, 0], masks[0])
                        nc.vector.tensor_mul(ex[:, 1], ex[:, 1], masks[1])
                    exs = asb.tile([128, 2, 256], BF16, tag="exsum")
                    nc.vector.tensor_add(exs, ex[:, 0], ex[:, 1])
                    return ex, exs

                pairs = [_simpair(0)]
                if t >= 1:
                    pairs.append(_simpair(1))
                box = [None, None]   # [carry exsum, chain accumulator]

                def _psd(p, exs):
                    # ONE denominator matmul per query tile: DVE chains the
                    # bf16 pair sums (positive values, <=7 adds — ~0.3%
                    # denominator rounding) so the ones-column stream hits
                    # the PE only once per tile
                    if P == 1:
                        nc.tensor.matmul(psd, ones_col, exs,
                                         start=True, stop=True)
                        return
                    if p == 0:
                        box[0] = exs
                        return
                    if p == 1:
                        acc = asb.tile([128, 2, 256], BF16, tag="dacc")
                        nc.vector.tensor_add(acc, box[0], exs)
                        box[1] = acc
                    else:
                        nc.vector.tensor_add(box[1], box[1], exs)
                    if p == P - 1:
                        nc.tensor.matmul(psd, ones_col, box[1],
                                         start=True, stop=True)

                for p in range(P):
                    ex, exs = pairs[p]
                    # the chain add must precede _simpair(p+2)'s DVE work:
                    # it is the reader that frees the exsum buffer the
                    # lookahead's pair-sum will claim (DVE executes in order)
                    _psd(p, exs)
                    if p + 2 < P:
                        pairs.append(_simpair(p + 2))
                    nc.tensor.matmul(psa, vnat[:, 2 * p, :], ex[:, 0],
                                     start=(p == 0), stop=False)
                    nc.tensor.matmul(psa, vnat[:, 2 * p + 1, :], ex[:, 1],
                                     start=False, stop=(p == P - 1))
                    # drain one pending outproj column-group: keeps the PE
                    # strictly busier than ACT's exp pace (p-state + no
                    # idle), instead of bunching them at tile boundaries
                    if oq:
                        oq.pop(0)()
                den = asb.tile([1, 2, 256], BF16, tag="den")
                with nc.allow_low_precision(reason="softmax denom recip bf16"):
                    nc.vector.reciprocal(den, psd)
                bc = asb.tile([128, 2, 256], BF16, tag="bc")
                nc.gpsimd.partition_broadcast(bc, den)
                nc.vector.tensor_mul(
                    attnT[:, h0:h1, t * 256:(t + 1) * 256], psa, bc)

            oq = []

            def _outproj(t, attnT=attnT):
                # enqueue the 8 column-group closures; the attention j-loops
                # drain them one per pair so PE backlog stays positive
                for m in (2 * t, 2 * t + 1):
                    ysb = outp.tile([128, DIM], BF16, tag="ysb")
                    for nso in range(DIM // 512):
                        def _grp(tail=False, m=m, nso=nso, ysb=ysb):
                            # in-loop drains use the dedicated 1-bank pool
                            # (spaced a pair apart, the copy hides); the
                            # back-to-back tail drains pipeline through the
                            # then-idle 2-buf ps_work pool instead
                            if tail:
                                psy = ps_work.tile([128, 512], F32, tag="pswork")
                            else:
                                psy = ps_out.tile([128, 512], F32, tag="psy")
                            for hc in range(HL):
                                nc.tensor.matmul(
                                    psy, attnT[:, hc, m * 128:(m + 1) * 128],
                                    wout_sb[:, hc, nso * 512:(nso + 1) * 512],
                                    start=(hc == 0), stop=(hc == HL - 1))
                            sl_y = ysb[:, nso * 512:(nso + 1) * 512]
                            with nc.allow_low_precision(reason="partial y bf16"):
                                if nso % 2 == 0:
                                    nc.vector.tensor_copy(sl_y, psy)
                                else:
                                    nc.scalar.copy(sl_y, psy)
                            if nso == DIM // 512 - 1:
                                nc.gpsimd.dma_start(
                                    y[m * 128:(m + 1) * 128, :], ysb)
                        oq.append(_grp)

            # ---- projections + rope, one n-slice at a time ----
            for ns in range(NS):
                sl = slice(ns * SL, (ns + 1) * SL)
                xt = xpool.tile([128, DC, SL], BF16, tag="xt")
                h_dc = DC // 2
                xt_src = xT.rearrange("(c p) n -> p c n", p=128)[:, :, sl]
                nc.sync.dma_start(xt[:, :h_dc, :], xt_src[:, :h_dc, :])
                nc.sync.dma_start(xt[:, h_dc:, :], xt_src[:, h_dc:, :])
                # v first: its psum->sbuf copy rides ACT so the PE transposes
                # aren't queued behind DVE rope work
                psv = ps_work.tile([128, SL], F32, tag="pswork")
                for dc in range(DC):
                    nc.tensor.matmul(
                        psv, wkv_sb[:, dc, D:2 * D], xt[:, dc, :],
                        start=(dc == 0), stop=(dc == DC - 1))
                vt_sb = sb.tile([128, SL], BF16, tag="vt")
                nc.scalar.copy(vt_sb, psv)
                # k BEFORE the q heads: the first attention tile (descending
                # order) reads krot of the LAST slice, so its rope must not
                # sit at the tail of the DVE queue at the region boundary;
                # q heads rope in 0..3 order, matching group-0-first reads
                psk = ps_work.tile([128, SL], F32, tag="pswork")
                for dc in range(DC):
                    nc.tensor.matmul(
                        psk, wkv_sb[:, dc, 0:D], xt[:, dc, :],
                        start=(dc == 0), stop=(dc == DC - 1))
                _rope(nc, sb, psk, krot[:, sl], ck_sb[:, sl], sk_sb[:, sl])
                for h in range(HL):
                    psq = ps_work.tile([128, SL], F32, tag="pswork")
                    for dc in range(DC):
                        nc.tensor.matmul(
                            psq, wq_sb[:, dc, h * D:(h + 1) * D], xt[:, dc, :],
                            start=(dc == 0), stop=(dc == DC - 1))
                    _rope(nc, sb, psq, qrot[:, h, sl], cq_sb[:, sl], sq_sb[:, sl])
                # v transposes last: vt_sb's ACT copy lands during the q/k mms
                for kc in range(KPS):
                    pst = ps_work.tile([128, 128], BF16, tag="pswork")
                    nc.tensor.transpose(pst, vt_sb[:, kc * 128:(kc + 1) * 128], ident)
                    nc.vector.tensor_copy(vnat[:, ns * KPS + kc, :], pst)

            # attention tiles run in DESCENDING t order: the small tiles
            # (shallow software pipeline, exp-latency-exposed) land at the
            # region end where drained outproj matmuls fill the PE stalls;
            # outproj(t) enqueues right after attn(t) and drains into the
            # following tiles' j-loops
            for t in range(NQT - 1, -1, -1):
                for g in range(GROUPS):
                    _attn_qtile(t, g)
                _outproj(t)
            while oq:
                oq.pop(0)(tail=True)

    nc.finalize()
    return nc


def make_host_inputs(x, Wq, Wkv, Wout, HL):
    """Shard + precompute per-core input maps (host side)."""
    B, N, DIM = x.shape
    bf = ml_dtypes.bfloat16
    xT = np.ascontiguousarray(x.transpose(0, 2, 1)).astype(bf)
    inv = 1.0 / (10000.0 ** (np.arange(0, D, 2, dtype=np.float64) / D))
    fr = np.arange(N, dtype=np.float64)[:, None] * inv[None, :]
    pos = np.concatenate([fr, fr], axis=-1)              # [N, D]
    cos_t = np.cos(pos).T.astype(np.float32)             # [D, N]
    sin_t = np.sin(pos).T.astype(np.float32)
    sign = np.ones((D, 1), np.float32)
    sign[:D // 2] = -1.0
    sin_r = sin_t * sign            # fold rotate_half's sign into the table
    shared = dict(
        wkv=Wkv.astype(bf),
        cosq=np.ascontiguousarray(cos_t * SCALE).astype(bf),
        sinq=np.ascontiguousarray(sin_r * SCALE).astype(bf),
        cosk=cos_t.astype(bf), sink=sin_r.astype(bf))
    in_maps = []
    n_slices = HEADS // HL
    for c in range(N_CORES):
        b = c // n_slices
        s = c % n_slices
        lo, hi = s * HL * D, (s + 1) * HL * D
        in_maps.append(dict(
            shared,
            xT=xT[b],
            wq=np.ascontiguousarray(Wq[:, lo:hi]).astype(bf),
            wout=np.ascontiguousarray(Wout[lo:hi, :]).astype(bf)))
    return in_maps


def kernel(x, Wq, Wkv, Wout):
    B, N, DIM = x.shape
    HL = HPC
    nc = build_nc(B, N, DIM, HL)
    in_maps = make_host_inputs(x, Wq, Wkv, Wout, HL)
    res = run_bass_kernel_spmd(nc, in_maps, core_ids=list(range(N_CORES)))
    y = np.zeros((B, N, DIM), np.float32)
    n_slices = HEADS // HL
    for c, r in enumerate(res.results):
        y[c // n_slices] += r["y"].astype(np.float32)
    return y


# revision 12
# speedup vs baseline: 1.2442x; 1.2442x over previous
"""Trainium2 Bass kernel for MQA causal attention (final, v11).

Sharding: hybrid batch x tensor-parallel (2 batches x 4 head-slices on 8
cores); shared K/V computed locally per batch; host sums 4 partial
out-projections per batch.

Schedule (everything serves keeping the in-order PE queue saturated):
  - attention processes key chunks in PAIRS: two sim matmuls land in one
    2-bank PSUM tile, ONE exp instruction covers both; sims issue 2 pairs
    ahead of their consumers so exp latency never blocks the PE
  - softmax denominator: bf16 pair-sums chain-accumulate on DVE, ONE
    ones-column matmul per query tile (the PE charges full moving-stream
    price regardless of output width)
  - causal masking via constant 0/1 mask tiles + DVE multiply
  - query tiles run in DESCENDING order; out-projection column-groups are
    enqueued per tile and drained one per key-chunk pair through a
    dedicated PSUM bank, so the PE holds backlog while ACT runs exp
    (p-state insurance); the residual queue drains through the then-idle
    2-buf work pool at rep end
  - rope in bf16 on DVE (2x mode); rotate-half happens in the ACT-side
    PSUM drain (partition-offset ops need a PSUM operand); per slice the
    k projection precedes the q heads so the region-boundary reader
    (krot, last slice) is never last in the DVE queue
  - partial y written in bf16 (halves output DMA; host upcasts and sums)
"""

import os
import sys
from contextlib import ExitStack

import numpy as np

for _p in ("/opt/trn_rl_repo",):
    if os.path.isdir(_p) and _p not in sys.path:
        sys.path.insert(0, _p)

import ml_dtypes

import concourse.bass as bass
import concourse.mybir as mybir
import concourse.tile as tile
from concourse import bacc
from concourse.bass_utils import run_bass_kernel_spmd
from concourse.masks import make_identity

HEADS = 16
D = 128
SCALE = D ** -0.5
N_CORES = 8
HPC = 4              # query heads per core (hybrid: 2 batches x 4 head slices)
GROUPS = HPC // 2    # attention processed in 2-head groups (PSUM budget)

F32 = mybir.dt.float32
BF16 = mybir.dt.bfloat16


def _rope(nc, sb_pool, ps, out_slice, cos_s, sin_s):
    """out_slice(bf16) = ps*cos_s + rot(ps)*sin_s. ACT performs the
    rotate-half during the PSUM drain (partition-offset copies are legal
    with a PSUM operand; SBUF-SBUF DVE ops must be partition-aligned), so
    the sin multiply and the add run in bf16 2x mode on DVE; sin_s arrives
    pre-signed from the host (rows 0-63 negated)."""
    L = ps.shape[-1]
    pr = sb_pool.tile([128, L], BF16, tag="ropepr")
    nc.scalar.copy(pr[0:64, :], ps[64:128, :])
    nc.scalar.copy(pr[64:128, :], ps[0:64, :])
    t1 = sb_pool.tile([128, L], BF16, tag="ropet1")
    nc.vector.tensor_mul(t1, ps, cos_s)
    t2 = sb_pool.tile([128, L], BF16, tag="ropet2")
    nc.vector.tensor_mul(t2, pr, sin_s)
    nc.vector.tensor_add(out_slice, t1, t2)


def build_nc(B, N, DIM, HL, reps=1):
    """One SPMD program: HL query heads + shared kv head, ONE batch."""
    DC = DIM // 128           # contraction chunks for projections
    SL = min(N, 512)          # projection n-slice length
    NS = N // SL              # n slices
    NKC = N // 128            # 128-wide key chunks
    NQT = N // 256            # 256-row query tiles
    KPS = SL // 128           # key chunks per slice

    nc = bacc.Bacc(None, target_bir_lowering=False)
    xT = nc.declare_dram_parameter("xT", [DIM, N], BF16, isOutput=False)
    wq = nc.declare_dram_parameter("wq", [DIM, HL * D], BF16, isOutput=False)
    wkv = nc.declare_dram_parameter("wkv", [DIM, 2 * D], BF16, isOutput=False)
    wout = nc.declare_dram_parameter("wout", [HL * D, DIM], BF16, isOutput=False)
    cosq = nc.declare_dram_parameter("cosq", [D, N], BF16, isOutput=False)
    sinq = nc.declare_dram_parameter("sinq", [D, N], BF16, isOutput=False)
    cosk = nc.declare_dram_parameter("cosk", [D, N], BF16, isOutput=False)
    sink = nc.declare_dram_parameter("sink", [D, N], BF16, isOutput=False)
    y = nc.declare_dram_parameter("y", [N, DIM], BF16, isOutput=True)

    with ExitStack() as ctx:
        tc = ctx.enter_context(tile.TileContext(nc))
        consts = ctx.enter_context(tc.tile_pool(name="consts", bufs=1))
        xpool = ctx.enter_context(tc.tile_pool(name="xpool", bufs=2))
        proj = ctx.enter_context(tc.tile_pool(name="proj", bufs=2))
        sb = ctx.enter_context(tc.tile_pool(name="sb", bufs=2))
        expool = ctx.enter_context(tc.tile_pool(name="expool", bufs=3))
        asb = ctx.enter_context(tc.tile_pool(name="asb", bufs=3))
        outp = ctx.enter_context(tc.tile_pool(name="outp", bufs=2))
        ps_work = ctx.enter_context(tc.tile_pool(name="ps_work", bufs=2, space="PSUM"))
        ps_att = ctx.enter_context(tc.tile_pool(name="ps_att", bufs=2, space="PSUM"))
        # psd is read (reciprocal) immediately after its stop, so one buf
        # suffices; the freed bank gives outproj psy its own pool so drained
        # column-groups never collide with the sim-pair buffer rotation
        ps_den = ctx.enter_context(tc.tile_pool(name="ps_den", bufs=1, space="PSUM"))
        ps_out = ctx.enter_context(tc.tile_pool(name="ps_out", bufs=1, space="PSUM"))

        ident = consts.tile([128, 128], BF16)
        make_identity(nc, ident)
        ones_col = consts.tile([128, 1], BF16)
        nc.vector.memset(ones_col, 1.0)
        # causal 0/1 masks for the two diagonal key chunks of each q tile:
        # chunk aligned with queries (keep q >= p) and chunk 128 past it
        masks = []
        for base in (0, -128):
            mk = consts.tile([128, 2, 256], BF16, name=f"mask{base}")
            nc.vector.memset(mk, 1.0)
            nc.gpsimd.affine_select(
                out=mk, in_=mk, compare_op=mybir.AluOpType.is_ge, fill=0.0,
                base=base, pattern=[[0, 2], [1, 256]], channel_multiplier=-1)
            masks.append(mk)

        wq_sb = consts.tile([128, DC, HL * D], BF16)
        wkv_sb = consts.tile([128, DC, 2 * D], BF16)
        nc.sync.dma_start(
            wq_sb, wq.rearrange("(c p) m -> p c m", p=128))
        nc.sync.dma_start(
            wkv_sb, wkv.rearrange("(c p) m -> p c m", p=128))
        wout_sb = consts.tile([128, HL, DIM], BF16)
        nc.scalar.dma_start(wout_sb, wout.rearrange("(c p) m -> p c m", p=128))
        cq_sb = consts.tile([128, N], BF16)
        sq_sb = consts.tile([128, N], BF16)
        ck_sb = consts.tile([128, N], BF16)
        sk_sb = consts.tile([128, N], BF16)
        nc.scalar.dma_start(cq_sb, cosq[:, :])
        nc.scalar.dma_start(sq_sb, sinq[:, :])
        nc.scalar.dma_start(ck_sb, cosk[:, :])
        nc.scalar.dma_start(sk_sb, sink[:, :])

        for rep in range(reps):
            qrot = proj.tile([128, HL, N], BF16, tag="qrot")
            krot = proj.tile([128, N], BF16, tag="krot")
            vnat = proj.tile([128, NKC, D], BF16, tag="vnat")
            attnT = proj.tile([128, HL, N], BF16, tag="attnT")

            def _attn_qtile(t, g, qrot=qrot, krot=krot, vnat=vnat, attnT=attnT):
                h0, h1 = 2 * g, 2 * g + 2
                P = t + 1                     # key-chunk pairs (2t+2 chunks)
                psa = ps_att.tile([128, 2, 256], F32, tag="psa")
                psd = ps_den.tile([1, 2, 256], F32, tag="psd")
                qsl = qrot[:, h0:h1, t * 256:(t + 1) * 256]

                # two sims + one exp + one pair-sum per PAIR of key chunks;
                # issued 2 pairs ahead of the consuming psd/psa matmuls so
                # the PE never waits on ACT in program order
                def _simpair(p):
                    pss = ps_work.tile([128, 2, 2, 256], F32, tag="pswork")
                    nc.tensor.matmul(
                        pss[:, 0], krot[:, (2 * p) * 128:(2 * p + 1) * 128],
                        qsl, start=True, stop=True)
                    nc.tensor.matmul(
                        pss[:, 1], krot[:, (2 * p + 1) * 128:(2 * p + 2) * 128],
                        qsl, start=True, stop=True)
                    ex = expool.tile([128, 2, 2, 256], BF16, tag="exp")
                    nc.scalar.activation(ex, pss, mybir.ActivationFunctionType.Exp)
                    if p == t:            # diagonal pair
                        nc.vector.tensor_mul(ex[:, 0], ex[:, 0], masks[0])
                        nc.vector.tensor_mul(ex[:, 1], ex[:, 1], masks[1])
                    exs = asb.tile([128, 2, 256], BF16, tag="exsum")
                    nc.vector.tensor_add(exs, ex[:, 0], ex[:, 1])
                    return ex, exs

                pairs = [_simpair(0)]
                if t >= 1:
                    pairs.append(_simpair(1))
                box = [None, None]   # [carry exsum, chain accumulator]

                def _psd(p, exs):
                    # ONE denominator matmul per query tile: DVE chains the
                    # bf16 pair sums (positive values, <=7 adds — ~0.3%
                    # denominator rounding) so the ones-column stream hits
                    # the PE only once per tile
                    if P == 1:
                        nc.tensor.matmul(psd, ones_col, exs,
                                         start=True, stop=True)
                        return
                    if p == 0:
                        box[0] = exs
                        return
                    if p == 1:
                        acc = asb.tile([128, 2, 256], BF16, tag="dacc")
                        nc.vector.tensor_add(acc, box[0], exs)
                        box[1] = acc
                    else:
                        nc.vector.tensor_add(box[1], box[1], exs)
                    if p == P - 1:
                        nc.tensor.matmul(psd, ones_col, box[1],
                                         start=True, stop=True)

                for p in range(P):
                    ex, exs = pairs[p]
                    # the chain add must precede _simpair(p+2)'s DVE work:
                    # it is the reader that frees the exsum buffer the
                    # lookahead's pair-sum will claim (DVE executes in order)
                    _psd(p, exs)
                    if p + 2 < P:
                        pairs.append(_simpair(p + 2))
                    nc.tensor.matmul(psa, vnat[:, 2 * p, :], ex[:, 0],
                                     start=(p == 0), stop=False)
                    nc.tensor.matmul(psa, vnat[:, 2 * p + 1, :], ex[:, 1],
                                     start=False, stop=(p == P - 1))
                    # drain one pending outproj column-group: keeps the PE
                    # strictly busier than ACT's exp pace (p-state + no
                    # idle), instead of bunching them at tile boundaries
                    if oq:
                        oq.pop(0)()
                den = asb.tile([1, 2, 256], BF16, tag="den")
                with nc.allow_low_precision(reason="softmax denom recip bf16"):
                    nc.vector.reciprocal(den, psd)
                bc = asb.tile([128, 2, 256], BF16, tag="bc")
                nc.gpsimd.partition_broadcast(bc, den)
                nc.vector.tensor_mul(
                    attnT[:, h0:h1, t * 256:(t + 1) * 256], psa, bc)

            oq = []

            def _outproj(t, attnT=attnT):
                # enqueue the 8 column-group closures; the attention j-loops
                # drain them one per pair so PE backlog stays positive
                for m in (2 * t, 2 * t + 1):
                    ysb = outp.tile([128, DIM], BF16, tag="ysb")
                    for nso in range(DIM // 512):
                        def _grp(tail=False, m=m, nso=nso, ysb=ysb):
                            # in-loop drains use the dedicated 1-bank pool
                            # (spaced a pair apart, the copy hides); the
                            # back-to-back tail drains pipeline through the
                            # then-idle 2-buf ps_work pool instead
                            if tail:
                                psy = ps_work.tile([128, 512], F32, tag="pswork")
                            else:
                                psy = ps_out.tile([128, 512], F32, tag="psy")
                            for hc in range(HL):
                                nc.tensor.matmul(
                                    psy, attnT[:, hc, m * 128:(m + 1) * 128],
                                    wout_sb[:, hc, nso * 512:(nso + 1) * 512],
                                    start=(hc == 0), stop=(hc == HL - 1))
                            sl_y = ysb[:, nso * 512:(nso + 1) * 512]
                            with nc.allow_low_precision(reason="partial y bf16"):
                                if nso % 2 == 0:
                                    nc.vector.tensor_copy(sl_y, psy)
                                else:
                                    nc.scalar.copy(sl_y, psy)
                            if nso == DIM // 512 - 1:
                                nc.gpsimd.dma_start(
                                    y[m * 128:(m + 1) * 128, :], ysb)
                        oq.append(_grp)

            # ---- projections + rope, one n-slice at a time ----
            for ns in range(NS):
                sl = slice(ns * SL, (ns + 1) * SL)
                xt = xpool.tile([128, DC, SL], BF16, tag="xt")
                h_dc = DC // 2
                xt_src = xT.rearrange("(c p) n -> p c n", p=128)[:, :, sl]
                nc.sync.dma_start(xt[:, :h_dc, :], xt_src[:, :h_dc, :])
                nc.sync.dma_start(xt[:, h_dc:, :], xt_src[:, h_dc:, :])
                # v first: its psum->sbuf copy rides ACT so the PE transposes
                # aren't queued behind DVE rope work
                psv = ps_work.tile([128, SL], F32, tag="pswork")
                for dc in range(DC):
                    nc.tensor.matmul(
                        psv, wkv_sb[:, dc, D:2 * D], xt[:, dc, :],
                        start=(dc == 0), stop=(dc == DC - 1))
                vt_sb = sb.tile([128, SL], BF16, tag="vt")
                nc.scalar.copy(vt_sb, psv)
                # k BEFORE the q heads: the first attention tile (descending
                # order) reads krot of the LAST slice, so its rope must not
                # sit at the tail of the DVE queue at the region boundary;
                # q heads rope in 0..3 order, matching group-0-first reads
                psk = ps_work.tile([128, SL], F32, tag="pswork")
                for dc in range(DC):
                    nc.tensor.matmul(
                        psk, wkv_sb[:, dc, 0:D], xt[:, dc, :],
                        start=(dc == 0), stop=(dc == DC - 1))
                _rope(nc, sb, psk, krot[:, sl], ck_sb[:, sl], sk_sb[:, sl])
                for h in range(HL):
                    psq = ps_work.tile([128, SL], F32, tag="pswork")
                    for dc in range(DC):
                        nc.tensor.matmul(
                            psq, wq_sb[:, dc, h * D:(h + 1) * D], xt[:, dc, :],
                            start=(dc == 0), stop=(dc == DC - 1))
                    _rope(nc, sb, psq, qrot[:, h, sl], cq_sb[:, sl], sq_sb[:, sl])
                # v transposes last: vt_sb's ACT copy lands during the q/k mms
                for kc in range(KPS):
                    pst = ps_work.tile([128, 128], BF16, tag="pswork")
                    nc.tensor.transpose(pst, vt_sb[:, kc * 128:(kc + 1) * 128], ident)
                    nc.vector.tensor_copy(vnat[:, ns * KPS + kc, :], pst)

            # attention tiles run in DESCENDING t order: the small tiles
            # (shallow software pipeline, exp-latency-exposed) land at the
            # region end where drained outproj matmuls fill the PE stalls;
            # outproj(t) enqueues right after attn(t) and drains into the
            # following tiles' j-loops
            for t in range(NQT - 1, -1, -1):
                for g in range(GROUPS):
                    _attn_qtile(t, g)
                _outproj(t)
            while oq:
                oq.pop(0)(tail=True)

    nc.finalize()
    return nc


def make_host_inputs(x, Wq, Wkv, Wout, HL):
    """Shard + precompute per-core input maps (host side)."""
    B, N, DIM = x.shape
    bf = ml_dtypes.bfloat16
    xT = np.ascontiguousarray(x.transpose(0, 2, 1)).astype(bf)
    inv = 1.0 / (10000.0 ** (np.arange(0, D, 2, dtype=np.float64) / D))
    fr = np.arange(N, dtype=np.float64)[:, None] * inv[None, :]
    pos = np.concatenate([fr, fr], axis=-1)              # [N, D]
    cos_t = np.cos(pos).T.astype(np.float32)             # [D, N]
    sin_t = np.sin(pos).T.astype(np.float32)
    sign = np.ones((D, 1), np.float32)
    sign[:D // 2] = -1.0
    sin_r = sin_t * sign            # fold rotate_half's sign into the table
    shared = dict(
        wkv=Wkv.astype(bf),
        cosq=np.ascontiguousarray(cos_t * SCALE).astype(bf),
        sinq=np.ascontiguousarray(sin_r * SCALE).astype(bf),
        cosk=cos_t.astype(bf), sink=sin_r.astype(bf))
    in_maps = []
    n_slices = HEADS // HL
    for c in range(N_CORES):
        b = c // n_slices
        s = c % n_slices
        lo, hi = s * HL * D, (s + 1) * HL * D
        in_maps.append(dict(
            shared,
            xT=xT[b],
            wq=np.ascontiguousarray(Wq[:, lo:hi]).astype(bf),
            wout=np.ascontiguousarray(Wout[lo:hi, :]).astype(bf)))
    return in_maps


def kernel(x, Wq, Wkv, Wout):
    B, N, DIM = x.shape
    HL = HPC
    nc = build_nc(B, N, DIM, HL)
    in_maps = make_host_inputs(x, Wq, Wkv, Wout, HL)
    res = run_bass_kernel_spmd(nc, in_maps, core_ids=list(range(N_CORES)))
    y = np.zeros((B, N, DIM), np.float32)
    n_slices = HEADS // HL
    for c, r in enumerate(res.results):
        y[c // n_slices] += r["y"].astype(np.float32)
    return y
